# revision 49
# baseline (speedup 1.0000x reference)
"""Multi-head attention + LayerNorm Trainium2 kernel (v3).

Full inputs: x [8, 1024, 512], Wq/Wk/Wv [512, 512], ln_gamma/ln_beta [512].
Data-parallel over batch: one batch element per NeuronCore (8 cores), no
collectives. Each core runs the identical single-core program below.

Per-core dataflow (S=1024 seq, E=512 emb, H=8 heads, D=64 head dim):
  1. PE-transpose x -> x^T and Wq/Wk/Wv -> W^T in plain fp32 (neuronxcc
     forbids mixing 32-bit and 16-bit matmul inputs, and f32r inputs must
     come from f32r-rounding producers). x^T/q^T/k^T/W^T are stored bf16
     so projection and QK matmuls run at 1 cyc/row at any moving width.
  2. q/k chunk-0 projections are emitted per-128-column region so the
     first QK scores tile only waits for x0-3 + Wq0/Wk0 and the ScalarE
     exp stream starts as early as the DMA bandwidth allows. Loads are
     split across the SP and Pool (SWDGE) DMA queues, with later W chunks
     throttled by the load-pool ring so they cannot jump ahead.
  3. Scores: scores_T[sk, sq] = kT.T @ qT per head (K=64, two heads
     row-tiled in one kT chunk), exp on ScalarE with the 1/sqrt(E) scale
     fused, reading PSUM directly (scores ~N(0, 0.35): no max needed).
     QK for pairs 0 AND 1 is emitted during phase 1 so the exp stream
     never starves while projections run; the pair loop then computes
     QK(p+2) alongside AV(p).
  4. AV in natural layout: o[sq, e] accumulates exp_T.T @ [v|1] with the
     128x128 exp tile stationary -> 65-column outputs, half the PE
     column-cycles of the transposed form and no U-transpose. AV trails
     QK by a 2-tk software-pipeline skew so the PSUM WAR on the po
     accumulators never blocks the QK/exp stream.
  5. Per-pair finalize: reciprocal of the normalizer column, divide into
     o_all, bn_stats per head pair. Pairs 2/3 run tq-major so divide /
     LayerNorm / DMA-out pipeline behind their AV matmuls. The single
     act-table switch (exp -> sqrt set) overlaps the endgame AV.
"""

import numpy as np
from contextlib import ExitStack

import concourse.bass as bass
import concourse.tile as tile
from concourse import bacc, mybir
from concourse.bass_utils import run_bass_kernel_spmd
from concourse.masks import make_identity

S = 1024
E = 512
H = 8
D = 64
P = 128
NE = E // P   # 4 e-chunks
NS = S // P   # 8 s-tiles
HP = H // 2   # 4 head pairs
DP1 = D + 1   # head dim + normalizer column
SCALE = float(E) ** -0.5
EPS = 1e-5
SKEW = 2      # AV trails QK by this many tk steps in the pair loop

F32 = mybir.dt.float32
F32R = mybir.dt.float32r
BF16 = mybir.dt.bfloat16
AF = mybir.ActivationFunctionType
ALU = mybir.AluOpType

PH = 66   # per-head stride in vext (64 v cols + 1 ones col + 1 pad)


def _emit(nc, tc, x_d, wq_d, wk_d, wv_d, g_d, b_d, out_d, apply_gb):
    ctx = ExitStack()
    with ctx:
        persist = ctx.enter_context(tc.tile_pool(name="persist", bufs=1))
        # all 32 exp tiles ([P,2,S] bf16) live in one ring; 24 bufs means
        # pair 3's 8 allocations recycle pair 0's slots (freed early by
        # AV(0)) instead of waiting on pair-1 tiles still being consumed
        exp0p = ctx.enter_context(tc.tile_pool(name="exp0", bufs=24))
        finp = ctx.enter_context(tc.tile_pool(name="fin", bufs=4))
        # one PSUM pool for the whole kernel: "w" is a 4-deep ring of 1-bank
        # tiles (transposes/projections in phase 1, po accumulators later --
        # the ring WAR doubles as the pair-to-pair po recycling), "sc" holds
        # two 2-bank score tiles so QK/exp pipeline across phases
        psp = ctx.enter_context(tc.tile_pool(name="psp", bufs=4, space="PSUM"))

        ident = persist.tile([P, P], F32, tag="ident", name="identx")
        make_identity(nc, ident)
        identb = persist.tile([P, P], BF16, tag="identb", name="identb")
        make_identity(nc, identb)
        eps_t = persist.tile([P, 1], F32, tag="eps", name="eps")
        nc.vector.memset(eps_t, EPS)
        scr = persist.tile([P, 1], F32, tag="scr", name="scr")
        if apply_gb:
            gam_b = persist.tile([P, E], F32, tag="gam", name="gam")
            nc.gpsimd.dma_start(out=gam_b, in_=g_d.partition_broadcast(P))
            bet_b = persist.tile([P, E], F32, tag="bet", name="bet")
            nc.gpsimd.dma_start(out=bet_b, in_=b_d.partition_broadcast(P))

        qkT = persist.tile([P, 2, NE, S], BF16, tag="qkT", name="qkT")
        qT = qkT[:, 0]
        kT = qkT[:, 1]
        xT = persist.tile([P, NE, S], BF16, tag="xT", name="xT")
        vext = persist.tile([P, NS, H * PH], BF16, tag="vext", name="vext")
        o_all = persist.tile([P, NS, E], F32, tag="o_all", name="o_all")
        st_all = persist.tile([P, NS, HP, 6], F32, tag="st_all", name="st_all")
        mvall = persist.tile([P, NS, 2], F32, tag="mvall", name="mvall")
        rs_all = persist.tile([P, NS], F32, tag="rs_all", name="rs_all")

        for t_i in range(NS):
            ones_v = vext[:, t_i, :].rearrange("p (h c) -> p h c", c=PH)[:, :, D:DP1]
            nc.gpsimd.memset(ones_v, 1.0)

        expp = exp0p
        exp_tiles = {}
        po_tiles = {}

        def qk_pair_tk(p, tk, pool):
            """Scores + exp for head pair p, sk tile tk; h-major so exp(h0)
            is unblocked after its own two matmuls."""
            for h in (2 * p, 2 * p + 1):
                sp = psp.tile([P, S], F32, tag="sc", bufs=2, name=f"sc{h}_{tk}")
                rows = slice((h % 2) * D, (h % 2) * D + D)
                for n in range(2):
                    nc.tensor.matmul(
                        out=sp[:, n * 512:(n + 1) * 512],
                        lhsT=kT[rows, p, tk * P:(tk + 1) * P],
                        rhs=qT[rows, p, n * 512:(n + 1) * 512],
                        start=True, stop=True,
                    )
                key = (h, tk // 2)
                if key not in exp_tiles:
                    exp_tiles[key] = pool.tile(
                        [P, 2, S], BF16, tag="exp", name=f"e{h}_{tk}"
                    )
                nc.scalar.activation(
                    out=exp_tiles[key][:, tk % 2, :], in_=sp, func=AF.Exp,
                    scale=SCALE,
                )

        def qk_half(p, tk, n, pool):
            """Half-sq scores + exp (fast-start: only needs q regions of
            one sq half)."""
            for h in (2 * p, 2 * p + 1):
                sp = psp.tile([P, E], F32, tag="sc", bufs=2,
                              name=f"sch{h}_{tk}_{n}")
                rows = slice((h % 2) * D, (h % 2) * D + D)
                nc.tensor.matmul(
                    out=sp,
                    lhsT=kT[rows, p, tk * P:(tk + 1) * P],
                    rhs=qT[rows, p, n * 512:(n + 1) * 512],
                    start=True, stop=True,
                )
                key = (h, tk // 2)
                if key not in exp_tiles:
                    exp_tiles[key] = pool.tile(
                        [P, 2, S], BF16, tag="exp", name=f"e{h}_{tk}"
                    )
                nc.scalar.activation(
                    out=exp_tiles[key][:, tk % 2, n * 512:(n + 1) * 512],
                    in_=sp, func=AF.Exp, scale=SCALE,
                )

        def alloc_po(p):
            po_tiles[p] = [
                psp.tile([P, 2, 2, DP1], F32, tag="w", bufs=4,
                         name=f"po{p}_{g}")
                for g in range(4)
            ]

        def av_step(p, tk):
            """16 natural-layout AV matmuls for pair p, sk tile tk."""
            pos = po_tiles[p]
            for hh, h in enumerate((2 * p, 2 * p + 1)):
                pair = exp_tiles[(h, tk // 2)]
                rhs = vext[:, tk, h * PH:h * PH + DP1]
                for g in range(4):
                    for j in range(2):
                        tq = 2 * g + j
                        nc.tensor.matmul(
                            out=pos[g][:, j, hh, :],
                            lhsT=pair[:, tk % 2, tq * P:(tq + 1) * P],
                            rhs=rhs,
                            start=(tk == 0), stop=(tk == NS - 1),
                        )

        def av_tq(p, g, j):
            """8-step AV accumulation chain for one (tq, head-pair)."""
            pos = po_tiles[p]
            tq = 2 * g + j
            for hh, h in enumerate((2 * p, 2 * p + 1)):
                for tk in range(NS):
                    nc.tensor.matmul(
                        out=pos[g][:, j, hh, :],
                        lhsT=exp_tiles[(h, tk // 2)][:, tk % 2,
                                                     tq * P:(tq + 1) * P],
                        rhs=vext[:, tk, h * PH:h * PH + DP1],
                        start=(tk == 0), stop=(tk == NS - 1),
                    )

        def finalize_pair_g(p, g, act_div=False):
            """Reciprocal of the normalizer column into SBUF, then divide.
            In-stream pairs use one broadcast-multiply per tq row on DVE
            (ACT is busy with exps); post-stream pairs split the divides
            ACT/DVE since ScalarE is idle there."""
            pos = po_tiles[p]
            rc = finp.tile([P, 2, 2], F32, tag="rc", bufs=8, name=f"rc{p}_{g}")
            nc.vector.reciprocal(out=rc, in_=pos[g][:, :, :, D:DP1])
            for j in range(2):
                tq = 2 * g + j
                if act_div:
                    nc.scalar.activation(
                        out=o_all[:, tq, 2 * p * D:(2 * p + 1) * D],
                        in_=pos[g][:, j, 0, 0:D], func=AF.Copy,
                        scale=rc[:, j, 0:1],
                    )
                    nc.vector.tensor_scalar_mul(
                        out=o_all[:, tq, (2 * p + 1) * D:(2 * p + 2) * D],
                        in0=pos[g][:, j, 1, 0:D],
                        scalar1=rc[:, j, 1:2],
                    )
                else:
                    nc.vector.tensor_tensor(
                        out=o_all[:, tq, 2 * p * D:(2 * p + 2) * D].rearrange(
                            "p (hh d) -> p hh d", d=D),
                        in0=pos[g][:, j, :, 0:D],
                        in1=rc[:, j, :, None].broadcast_to([P, 2, D]),
                        op=ALU.mult,
                    )
                nc.vector.bn_stats(
                    out=st_all[:, tq, p, :],
                    in_=o_all[:, tq, 2 * p * D:(2 * p + 2) * D],
                )

        # ---- Phase 1: loads, transposes, projections, QK pairs 0+1 ------
        # phase-1 PSUM pool: the po accumulators are not needed yet, so all
        # 8 banks go to deep transpose/proj (4x 1-bank) + score (2x 2-bank)
        # rings instead of the 2-slot ring the window phase uses
        with tc.tile_pool(name="wTp", bufs=1) as wT_pool, \
             tc.tile_pool(name="ldx", bufs=6) as ldx, \
             tc.tile_pool(name="ldw", bufs=2) as ldw, \
             tc.tile_pool(name="wb", bufs=4) as wbp:
            wT = wT_pool.tile([P, 3 * NE, E], BF16, tag="wT", name="wT")

            # x tiles on the SP queue; all W chunks on the Pool (SWDGE)
            # queue, where the 4-slot ldw ring throttles chunks 2/3 and Wv
            # behind the transposes of earlier chunks so they cannot jump
            # ahead of the x stream on the shared DMA engines.
            xnat = []
            for t_i in range(NS):
                xload = ldx.tile([P, E], F32, name="xload")
                nc.sync.dma_start(out=xload, in_=x_d[t_i * P:(t_i + 1) * P, :])
                xnat.append(xload)
            wnat = {}

            def load_w(wi, c, pool=None):
                # later W chunks ride the ldx ring: its slot WAR (freed by
                # x-tile transposes) sequences their transfers behind the
                # x stream on the shared DMA engines
                w_d = (wq_d, wk_d, wv_d)[wi]
                wload = (pool or ldx).tile([P, E], F32, name="wload")
                nc.gpsimd.dma_start(out=wload, in_=w_d[c * P:(c + 1) * P, :])
                wnat[(wi, c)] = wload

            load_w(0, 0, ldw)
            load_w(1, 0, ldw)
            load_w(0, 1)
            load_w(1, 1)

            def x_transpose_tile(t_i):
                """Transpose x tile t into xT[:, :, t*P:(t+1)*P] (bf16)."""
                pt = psp.tile([P, NE, P], F32, tag="w", name=f"psx{t_i}")
                for ce in range(NE):
                    nc.tensor.matmul(
                        out=pt[:, ce, :],
                        lhsT=xnat[t_i][:, ce * P:(ce + 1) * P],
                        rhs=ident,
                        is_transpose=True,
                    )
                nc.vector.tensor_copy(
                    out=xT[:, :, t_i * P:(t_i + 1) * P], in_=pt
                )

            def w_transpose_group(wi, cs, on_act=False, bf=False):
                """Transpose W row-chunk cs into column-block cs of all four
                W^T chunks (proj chunk c_out only needs group cs == c_out).
                bf=True: convert to bf16 on the idle Pool engine first, then
                transpose at 1 cyc/row with the bf16 identity."""
                if bf:
                    wb = wbp.tile([P, E], BF16, tag="wb", name=f"wb{wi}_{cs}")
                    nc.gpsimd.tensor_copy(out=wb, in_=wnat[(wi, cs)])
                    pt = psp.tile([P, NE, P], BF16, tag="w",
                                  name=f"psw{wi}_{cs}")
                    for ce in range(NE):
                        nc.tensor.matmul(
                            out=pt[:, ce, :],
                            lhsT=wb[:, ce * P:(ce + 1) * P],
                            rhs=identb,
                            is_transpose=True,
                        )
                else:
                    pt = psp.tile([P, NE, P], F32, tag="w", name=f"psw{wi}_{cs}")
                    for ce in range(NE):
                        nc.tensor.matmul(
                            out=pt[:, ce, :],
                            lhsT=wnat[(wi, cs)][:, ce * P:(ce + 1) * P],
                            rhs=ident,
                            is_transpose=True,
                        )
                dst = wT[:, wi * NE:(wi + 1) * NE, cs * P:(cs + 1) * P]
                src = pt
                if on_act:
                    nc.scalar.copy(out=dst, in_=src)
                else:
                    nc.vector.tensor_copy(out=dst, in_=src)

            def proj_region(t_i, on_act=False):
                """Chunk-0 q AND k projection for s columns of x tile t in
                one 1-bank psum tile + one copy: halves the "w"-ring users
                that were serializing phase 1's second half."""
                pp = psp.tile([P, 2, P], F32, tag="w", name=f"pr{t_i}")
                for wi in range(2):
                    for ce in range(NE):
                        nc.tensor.matmul(
                            out=pp[:, wi, :],
                            lhsT=wT[:, wi * NE + ce, 0:P],
                            rhs=xT[:, ce, t_i * P:(t_i + 1) * P],
                            start=(ce == 0), stop=(ce == NE - 1),
                        )
                d = qkT[:, :, 0, t_i * P:(t_i + 1) * P]
                if on_act:
                    nc.scalar.copy(out=d, in_=pp)
                else:
                    nc.vector.tensor_copy(out=d, in_=pp)

            def proj_half(c_out, wi, dst, n, on_act=False):
                pp = psp.tile([P, E], F32, tag="w",
                              name=f"pph{wi}_{c_out}_{n}")
                for ce in range(NE):
                    nc.tensor.matmul(
                        out=pp,
                        lhsT=wT[:, wi * NE + ce, c_out * P:(c_out + 1) * P],
                        rhs=xT[:, ce, n * 512:(n + 1) * 512],
                        start=(ce == 0), stop=(ce == NE - 1),
                    )
                d = dst[:, c_out, n * 512:(n + 1) * 512]
                if on_act:
                    nc.scalar.copy(out=d, in_=pp)
                else:
                    nc.vector.tensor_copy(out=d, in_=pp)

            def v_proj_tile(t_i):
                pv = psp.tile([P, E], F32, tag="w", name=f"pv{t_i}")
                for ce in range(NE):
                    nc.tensor.matmul(
                        out=pv,
                        lhsT=xT[:, ce, t_i * P:(t_i + 1) * P],
                        rhs=wT[:, 2 * NE + ce, :],
                        start=(ce == 0), stop=(ce == NE - 1),
                    )
                vdst = vext[:, t_i, :].rearrange("p (h c) -> p h c", c=PH)[:, :, 0:D]
                nc.vector.tensor_copy(out=vdst, in_=pv)

            # fast start: per-tile transposes + chunk-0 q/k regions so the
            # first scores tile waits only for x0-3 + Wq0/Wk0
            for t_i in range(4):
                x_transpose_tile(t_i)
            w_transpose_group(0, 0)
            w_transpose_group(1, 0, on_act=True)
            for t_i in range(4):
                proj_region(t_i, on_act=(t_i % 2 == 1))
            qk_half(0, 0, 0, exp0p)
            qk_half(0, 1, 0, exp0p)
            for t_i in range(4, NS):
                x_transpose_tile(t_i)
                proj_region(t_i)
            qk_half(0, 2, 0, exp0p)
            qk_half(0, 3, 0, exp0p)
            qk_half(0, 0, 1, exp0p)
            qk_half(0, 1, 1, exp0p)
            qk_half(0, 2, 1, exp0p)
            qk_half(0, 3, 1, exp0p)

            # chunk-1 projections, then full-width QK for pair-0 tk 4-7
            w_transpose_group(0, 1, bf=True)
            w_transpose_group(1, 1, bf=True)
            proj_half(1, 0, qT, 0)
            qk_pair_tk(0, 4, exp0p)
            proj_half(1, 1, kT, 0)
            qk_pair_tk(0, 5, exp0p)
            proj_half(1, 0, qT, 1)
            qk_pair_tk(0, 6, exp0p)
            proj_half(1, 1, kT, 1)
            qk_pair_tk(0, 7, exp0p)

            # chunk 2/3 W transposes + chunk-2 projections; Wv loads reuse
            # the throttled ldw ring
            load_w(0, 2)
            load_w(1, 2)
            load_w(0, 3)
            load_w(1, 3)
            w_transpose_group(0, 2, bf=True)
            w_transpose_group(1, 2, bf=True)
            qk_pair_tk(1, 0, expp)
            proj_half(2, 0, qT, 0)
            qk_pair_tk(1, 1, expp)
            proj_half(2, 1, kT, 0)
            qk_pair_tk(1, 2, expp)
            proj_half(2, 0, qT, 1)
            qk_pair_tk(1, 3, expp)
            proj_half(2, 1, kT, 1)
            w_transpose_group(0, 3, bf=True)
            w_transpose_group(1, 3, bf=True)
            for c in range(NE):
                load_w(2, c)
            qk_pair_tk(1, 4, expp)
            proj_half(3, 0, qT, 0)
            proj_half(3, 1, kT, 0)
            qk_pair_tk(1, 5, expp)
            proj_half(3, 0, qT, 1)
            proj_half(3, 1, kT, 1)
            w_transpose_group(2, 0, bf=True)
            w_transpose_group(2, 1, bf=True)
            qk_pair_tk(1, 6, expp)
            w_transpose_group(2, 2, bf=True)
            w_transpose_group(2, 3, bf=True)
            v_proj_tile(0)
            v_proj_tile(1)
            qk_pair_tk(1, 7, expp)
            qk_pair_tk(2, 0, expp)
            v_proj_tile(2)
            v_proj_tile(3)
            qk_pair_tk(2, 1, expp)
            for t_i in range(4, NS):
                v_proj_tile(t_i)

        # ---- Phase 2: QK(2)+QK(3) stream with g-major AV riding behind --
        # AV(p, g) needs only exps(p) (complete one section earlier) and a
        # po ring slot (freed by finalize(p-1, g)); the divides stay on DVE
        # because ACT is still streaming exps here
        alloc_po(0)
        for g in range(4):
            av_tq(0, g, 0)
            if 2 + 2 * g < NS:
                qk_pair_tk(2, 2 + 2 * g, expp)
            av_tq(0, g, 1)
            if 3 + 2 * g < NS:
                qk_pair_tk(2, 3 + 2 * g, expp)
            finalize_pair_g(0, g)
        alloc_po(1)
        for g in range(4):
            av_tq(1, g, 0)
            qk_pair_tk(3, 2 * g, expp)
            av_tq(1, g, 1)
            qk_pair_tk(3, 2 * g + 1, expp)
            finalize_pair_g(1, g)

        # pre-switch the ACT table to the sqrt set; reading the last exp
        # tile pins this after the exp stream so the scheduler cannot hoist
        # it (and its table load) ahead of the exps
        nc.scalar.activation(
            out=scr, in_=exp_tiles[(H - 1, NS // 2 - 1)][:, 1, 0:1],
            func=AF.Sqrt,
        )

        # ---- Phase 3: pairs 2+3, finalize + LN pipelined per g ----------
        alloc_po(2)
        for g in range(4):
            av_tq(2, g, 0)
            av_tq(2, g, 1)
            finalize_pair_g(2, g)
        alloc_po(3)
        pos = po_tiles[3]
        for g in range(4):
            for j in range(2):
                tq = 2 * g + j
                av_tq(3, g, j)
                xc = finp.tile([P, E], F32, tag="xc", bufs=8, name=f"xc{tq}")
                sd = finp.tile([P, 1], F32, tag="sd", bufs=8, name=f"sd{tq}")
                rsd = finp.tile([P, 1], F32, tag="rsd", bufs=8, name=f"rsd{tq}")
                rc = finp.tile([P, 2], F32, tag="rcl", bufs=8, name=f"rcl{tq}")
                nc.vector.reciprocal(out=rc, in_=pos[g][:, j, :, D:DP1])
                if j == 1:
                    # odd tq: divides on post-stream ScalarE so DVE stays
                    # under the 728ns/store DMA rate
                    nc.scalar.activation(
                        out=o_all[:, tq, 6 * D:7 * D],
                        in_=pos[g][:, j, 0, 0:D], func=AF.Copy,
                        scale=rc[:, 0:1],
                    )
                    nc.scalar.activation(
                        out=o_all[:, tq, 7 * D:8 * D],
                        in_=pos[g][:, j, 1, 0:D], func=AF.Copy,
                        scale=rc[:, 1:2],
                    )
                else:
                    nc.vector.tensor_tensor(
                        out=o_all[:, tq, 6 * D:8 * D].rearrange(
                            "p (hh d) -> p hh d", d=D),
                        in0=pos[g][:, j, :, 0:D],
                        in1=rc[:, :, None].broadcast_to([P, 2, D]),
                        op=ALU.mult,
                    )
                nc.vector.bn_stats(
                    out=st_all[:, tq, 3, :],
                    in_=o_all[:, tq, 6 * D:8 * D],
                )
                nc.vector.bn_aggr(out=mvall[:, tq, :], in_=st_all[:, tq, :, :])
                nc.scalar.activation(
                    out=sd, in_=mvall[:, tq, 1:2], func=AF.Sqrt,
                    bias=eps_t,
                )
                nc.vector.reciprocal(out=rsd, in_=sd)
                eng = nc.gpsimd if j == 0 else nc.vector
                eng.tensor_scalar(
                    out=xc, in0=o_all[:, tq, :],
                    scalar1=mvall[:, tq, 0:1],
                    scalar2=rsd,
                    op0=ALU.subtract, op1=ALU.mult,
                )
                if apply_gb:
                    nc.vector.tensor_mul(out=xc, in0=xc, in1=gam_b)
                    nc.vector.tensor_add(out=xc, in0=xc, in1=bet_b)
                nc.sync.dma_start(
                    out=out_d[tq * P:(tq + 1) * P, :], in_=xc,
                )


def build_attention(apply_gb=True):
    nc = bacc.Bacc("TRN2", target_bir_lowering=False, debug=False)
    x_d = nc.dram_tensor("x", [S, E], F32, kind="ExternalInput").ap()
    wq_d = nc.dram_tensor("Wq", [E, E], F32, kind="ExternalInput").ap()
    wk_d = nc.dram_tensor("Wk", [E, E], F32, kind="ExternalInput").ap()
    wv_d = nc.dram_tensor("Wv", [E, E], F32, kind="ExternalInput").ap()
    g_d = nc.dram_tensor("ln_gamma", [E], F32, kind="ExternalInput").ap()
    b_d = nc.dram_tensor("ln_beta", [E], F32, kind="ExternalInput").ap()
    out_d = nc.dram_tensor("out", [S, E], F32, kind="ExternalOutput").ap()
    with tile.TileContext(nc) as tc:
        _emit(nc, tc, x_d, wq_d, wk_d, wv_d, g_d, b_d, out_d, apply_gb)
    nc.compile()
    return nc


_CACHE = {}


def _get_nc(apply_gb=True):
    key = ("nc", apply_gb)
    if key not in _CACHE:
        _CACHE[key] = build_attention(apply_gb)
    return _CACHE[key]


def kernel(x, Wq, Wk, Wv, ln_gamma, ln_beta):
    g = np.ascontiguousarray(ln_gamma, dtype=np.float32)
    b = np.ascontiguousarray(ln_beta, dtype=np.float32)
    apply_gb = not (np.all(g == 1.0) and np.all(b == 0.0))
    nc = _get_nc(apply_gb)
    B = x.shape[0]
    wq = np.ascontiguousarray(Wq, dtype=np.float32)
    wk = np.ascontiguousarray(Wk, dtype=np.float32)
    wv = np.ascontiguousarray(Wv, dtype=np.float32)
    in_maps = [
        {
            "x": np.ascontiguousarray(x[i], dtype=np.float32),
            "Wq": wq, "Wk": wk, "Wv": wv,
            "ln_gamma": g, "ln_beta": b,
        }
        for i in range(B)
    ]
    try:
        res = run_bass_kernel_spmd(nc, in_maps, core_ids=list(range(B)))
    except Exception:
        # transient accelerator failures (e.g. NRT_EXEC_UNIT_UNRECOVERABLE
        # after a prior run wedged the device) usually clear on retry
        import time as _time
        _time.sleep(30)
        res = run_bass_kernel_spmd(nc, in_maps, core_ids=list(range(B)))
    return np.stack([res.results[i]["out"] for i in range(B)], axis=0)
